# revision 10
# baseline (speedup 1.0000x reference)
"""MoE (top-2 routing, SwiGLU experts + shared expert) on 8 TRN2 NeuronCores.

Strategy: token-parallel across cores (2048 tokens/core), experts replicated.
Per core, entirely on device:
  P1 router: fp32 scores = sigmoid(x @ gate_w^T), top-2 via DVE max8/max_index,
     gate normalization, slot assignment via matmul-cumsum (triangular-ones
     matmuls) into a per-(core,expert) capacity buffer (128 slots/expert),
     dispatch = indirect row-scatter of gate-scaled bf16 token rows into xb.
  P2 expert FFN: for each of 64 experts, transpose-load its 128 xb rows,
     bf16 matmuls silu(x@w1^T)*(x@w3^T) @ w2^T -> ob rows (token-major).
  P3 shared expert: same FFN on natural token tiles, result resident in SBUF.
  P4 combine: indirect row-gather of each token's two expert output rows,
     out = gathered1 + gathered2 + shared.
No collectives; host only slices/casts/concatenates.
"""

import numpy as np
import ml_dtypes
from contextlib import ExitStack

import concourse.bass as bass
from concourse import bacc
import concourse.mybir as mybir
import concourse.tile as tile
from concourse.bass import ts, ds, IndirectOffsetOnAxis
from concourse import bass_utils

P = 128
NCORES = 8
N, D, H, E = 16384, 1024, 512, 64
TPC = N // NCORES        # 2048 tokens per core
NT = TPC // P            # 16 token tiles per core
DJ = D // P              # 8 contraction chunks over D
HJ = H // P              # 4 chunks over H
CAP = 128                # per-core per-expert slot capacity (1 tile)
NSLOT = E * CAP          # 8192
BIG = 1.0e7
SIM_SILU = False

BF = mybir.dt.bfloat16
F32 = mybir.dt.float32
I32 = mybir.dt.int32
U32 = mybir.dt.uint32
AX = mybir.AxisListType.X
OP = mybir.AluOpType
ACTF = mybir.ActivationFunctionType


def ffn_tile(nc, xT, w1sb, w3sb, w2sb, hT, ps_h, ps_ob, ob_dst_f32=None,
             ob_sb=None):
    """SwiGLU FFN for one 128-token tile.

    xT:   [P, DJ, P] bf16 (D on partitions, tokens on free)
    w1sb/w3sb: [P, DJ, H] bf16 (lhsT blocks, d on partitions, h on free)
    w2sb: [P, HJ, D] bf16 (h on partitions, d on free)
    hT:   [P, HJ, P] bf16 scratch tile (h on partitions, tokens free)
    writes token-major [P, D] output into ob_sb (bf16 tile slice).
    """
    for j in range(HJ):
        h1 = ps_h.tile([P, P], F32, tag="h1")
        h3 = ps_h.tile([P, P], F32, tag="h3")
        for i in range(DJ):
            nc.tensor.matmul(out=h1[:], lhsT=w1sb[:, i, ts(j, P)], rhs=xT[:, i, :],
                             start=(i == 0), stop=(i == DJ - 1))
        for i in range(DJ):
            nc.tensor.matmul(out=h3[:], lhsT=w3sb[:, i, ts(j, P)], rhs=xT[:, i, :],
                             start=(i == 0), stop=(i == DJ - 1))
        s1 = ps_h.pool_sb.tile([P, P], F32, tag="silu")
        if SIM_SILU:  # CoreSim has no Silu; emulate via sigmoid * x
            nc.scalar.activation(s1[:], h1[:], ACTF.Sigmoid)
            nc.vector.tensor_mul(out=s1[:], in0=s1[:], in1=h1[:])
        else:
            nc.scalar.activation(s1[:], h1[:], ACTF.Silu)
        nc.vector.tensor_mul(out=hT[:, j, :], in0=s1[:], in1=h3[:])
    for nh in range(2):
        obps = ps_ob.tile([P, D // 2], F32, tag="ob")
        for j in range(HJ):
            nc.tensor.matmul(out=obps[:], lhsT=hT[:, j, :],
                             rhs=w2sb[:, j, ds(nh * (D // 2), D // 2)],
                             start=(j == 0), stop=(j == HJ - 1))
        nc.vector.tensor_copy(out=ob_sb[:, ds(nh * (D // 2), D // 2)], in_=obps[:])


def build_bass():
    nc = bacc.Bacc("TRN2", target_bir_lowering=False)
    # ---- I/O ----
    xt32 = nc.dram_tensor("xt32", [D, TPC], F32, kind="ExternalInput")
    xbf = nc.dram_tensor("xbf", [TPC, D], BF, kind="ExternalInput")
    xtbf = nc.dram_tensor("xtbf", [D, TPC], BF, kind="ExternalInput")
    gwt = nc.dram_tensor("gwt", [D, E], F32, kind="ExternalInput")
    w1t = nc.dram_tensor("w1t", [E, D, H], BF, kind="ExternalInput")
    w3t = nc.dram_tensor("w3t", [E, D, H], BF, kind="ExternalInput")
    w2t = nc.dram_tensor("w2t", [E, H, D], BF, kind="ExternalInput")
    w1st = nc.dram_tensor("w1st", [D, H], BF, kind="ExternalInput")
    w3st = nc.dram_tensor("w3st", [D, H], BF, kind="ExternalInput")
    w2st = nc.dram_tensor("w2st", [H, D], BF, kind="ExternalInput")
    biasb = nc.dram_tensor("biasb", [P, E], F32, kind="ExternalInput")
    iotab = nc.dram_tensor("iotab", [P, E], F32, kind="ExternalInput")
    ebasem1 = nc.dram_tensor("ebasem1", [P, E], F32, kind="ExternalInput")
    triu = nc.dram_tensor("triu", [P, P], F32, kind="ExternalInput")
    trils = nc.dram_tensor("trils", [P, P], F32, kind="ExternalInput")
    out = nc.dram_tensor("out", [TPC, D], F32, kind="ExternalOutput")
    xb = nc.dram_tensor("xb", [NSLOT, D], BF, kind="Internal")
    ob = nc.dram_tensor("ob", [NSLOT, D], BF, kind="Internal")

    xt32_r = xt32.rearrange("(a p) t -> p a t", p=P)
    xtbf_r = xtbf.rearrange("(a p) t -> p a t", p=P)
    gwt_r = gwt.rearrange("(a p) e -> p a e", p=P)

    with ExitStack() as ctx:
        tc = ctx.enter_context(tile.TileContext(nc))
        const = ctx.enter_context(tc.tile_pool(name="const", bufs=1))
        swpool = ctx.enter_context(tc.tile_pool(name="sw", bufs=1))
        spool = ctx.enter_context(tc.tile_pool(name="sres", bufs=1))
        wpool = ctx.enter_context(tc.tile_pool(name="wstream", bufs=2))
        xpool = ctx.enter_context(tc.tile_pool(name="xtiles", bufs=2))
        rpool = ctx.enter_context(tc.tile_pool(name="router", bufs=2))
        hpool = ctx.enter_context(tc.tile_pool(name="hsb", bufs=3))
        obpool = ctx.enter_context(tc.tile_pool(name="obsb", bufs=3))
        cpool = ctx.enter_context(tc.tile_pool(name="combine", bufs=2))
        ps_r = ctx.enter_context(tc.tile_pool(name="ps_r", bufs=1, space="PSUM"))
        ps_cs = ctx.enter_context(tc.tile_pool(name="ps_cs", bufs=1, space="PSUM"))
        ps_h = ctx.enter_context(tc.tile_pool(name="ps_h", bufs=2, space="PSUM"))
        ps_ob = ctx.enter_context(tc.tile_pool(name="ps_ob", bufs=2, space="PSUM"))
        ps_h.pool_sb = hpool  # convenience for ffn_tile silu scratch

        # ---- consts & resident tensors ----
        gw_sb = const.tile([P, DJ, E], F32)
        nc.sync.dma_start(gw_sb[:], gwt_r[:])
        bias_sb = const.tile([P, E], F32)
        nc.sync.dma_start(bias_sb[:], biasb[:])
        iota_sb = const.tile([P, E], F32)
        nc.sync.dma_start(iota_sb[:], iotab[:])
        ebase_sb = const.tile([P, E], F32)
        nc.sync.dma_start(ebase_sb[:], ebasem1[:])
        triu_sb = const.tile([P, P], F32)
        nc.sync.dma_start(triu_sb[:], triu[:])
        trils_sb = const.tile([P, P], F32)
        nc.sync.dma_start(trils_sb[:], trils[:])

        w1s_sb = swpool.tile([P, DJ, H], BF)
        nc.sync.dma_start(w1s_sb[:], w1st.rearrange("(a p) h -> p a h", p=P))
        w3s_sb = swpool.tile([P, DJ, H], BF)
        nc.sync.dma_start(w3s_sb[:], w3st.rearrange("(a p) h -> p a h", p=P))
        w2s_sb = swpool.tile([P, HJ, D], BF)
        nc.sync.dma_start(w2s_sb[:], w2st.rearrange("(j p) d -> p j d", p=P))

        S_sb = spool.tile([P, NT, D], BF)          # shared-expert outputs, resident
        slots_sb = spool.tile([P, NT, 2], F32)     # slot ids per token per pick

        bnd_reg = nc.gpsimd.alloc_register("bnd")
        nc.gpsimd.reg_mov(bnd_reg, NSLOT - 1)

        # ---- zero-fill xb (pad rows must be finite) ----
        zeros_sb = const.tile([P, 4, D], BF)
        nc.vector.memset(zeros_sb[:], 0.0)
        for c in range(NSLOT // 512):
            nc.gpsimd.dma_start(
                xb[ts(c, 512), :].rearrange("(q p) d -> p q d", p=P),
                zeros_sb[:],
            )

        # ================= P1: router + slot assignment + dispatch ============
        csps = ps_cs.tile([P, E], F32)  # running cumsum psum, persists across tiles
        for t in range(NT):
            xt_sb = rpool.tile([P, DJ, P], F32, tag="xt32")
            nc.sync.dma_start(xt_sb[:], xt32_r[:, :, ts(t, P)])
            scps = ps_r.tile([P, E], F32, tag="scores")
            for i in range(DJ):
                nc.tensor.matmul(out=scps[:], lhsT=xt_sb[:, i, :], rhs=gw_sb[:, i, :],
                                 start=(i == 0), stop=(i == DJ - 1))
            scores = rpool.tile([P, E], F32, tag="scores_sb")
            nc.scalar.activation(scores[:], scps[:], ACTF.Sigmoid)
            sel = rpool.tile([P, E], F32, tag="sel")
            nc.vector.tensor_add(out=sel[:], in0=scores[:], in1=bias_sb[:])
            mx = rpool.tile([P, 8], F32, tag="mx")
            nc.vector.max(out=mx[:], in_=sel[:])
            mxi = rpool.tile([P, 8], U32, tag="mxi")
            nc.vector.max_index(out=mxi[:], in_max=mx[:], in_values=sel[:])
            idxf = rpool.tile([P, 2], F32, tag="idxf")
            nc.vector.tensor_copy(out=idxf[:], in_=mxi[:, 0:2])
            oh1 = rpool.tile([P, E], F32, tag="oh1")
            nc.vector.tensor_scalar(oh1[:], iota_sb[:], idxf[:, 0:1], None,
                                    op0=OP.is_equal)
            oh2 = rpool.tile([P, E], F32, tag="oh2")
            nc.vector.tensor_scalar(oh2[:], iota_sb[:], idxf[:, 1:2], None,
                                    op0=OP.is_equal)
            # raw scores at the two picks; normalized gates
            tmp = rpool.tile([P, E], F32, tag="tmp")
            nc.vector.tensor_mul(out=tmp[:], in0=scores[:], in1=oh1[:])
            val1 = rpool.tile([P, 1], F32, tag="val1")
            nc.vector.reduce_sum(out=val1[:], in_=tmp[:], axis=AX)
            nc.vector.tensor_mul(out=tmp[:], in0=scores[:], in1=oh2[:])
            val2 = rpool.tile([P, 1], F32, tag="val2")
            nc.vector.reduce_sum(out=val2[:], in_=tmp[:], axis=AX)
            den = rpool.tile([P, 1], F32, tag="den")
            nc.vector.tensor_add(out=den[:], in0=val1[:], in1=val2[:])
            nc.vector.tensor_scalar_add(den[:], den[:], 1e-20)
            rec = rpool.tile([P, 1], F32, tag="rec")
            nc.vector.reciprocal(rec[:], den[:])
            g1 = rpool.tile([P, 1], F32, tag="g1")
            nc.vector.tensor_mul(out=g1[:], in0=val1[:], in1=rec[:])
            g2 = rpool.tile([P, 1], F32, tag="g2")
            nc.vector.tensor_mul(out=g2[:], in0=val2[:], in1=rec[:])

            # cumulative per-expert rank (inclusive), then convert the psum to
            # column totals for the next tile by adding strictly-lower part.
            oh = rpool.tile([P, E], F32, tag="ohsum")
            nc.vector.tensor_add(out=oh[:], in0=oh1[:], in1=oh2[:])
            nc.tensor.matmul(out=csps[:], lhsT=triu_sb[:], rhs=oh[:],
                             start=(t == 0), stop=False, skip_group_check=True)
            # slot = e*CAP + (incl-1) if incl <= CAP else BIG
            valid = rpool.tile([P, E], F32, tag="valid")
            nc.vector.tensor_scalar(valid[:], csps[:], float(CAP), None, op0=OP.is_le)
            slotm = rpool.tile([P, E], F32, tag="slotm")
            nc.vector.tensor_add(out=slotm[:], in0=csps[:], in1=ebase_sb[:])
            nc.vector.tensor_scalar_add(slotm[:], slotm[:], -BIG)
            nc.vector.tensor_mul(out=slotm[:], in0=slotm[:], in1=valid[:])
            nc.vector.tensor_scalar_add(slotm[:], slotm[:], BIG)
            nc.vector.tensor_mul(out=tmp[:], in0=slotm[:], in1=oh1[:])
            nc.vector.reduce_sum(out=slots_sb[:, t, 0:1], in_=tmp[:], axis=AX)
            nc.vector.tensor_mul(out=tmp[:], in0=slotm[:], in1=oh2[:])
            nc.vector.reduce_sum(out=slots_sb[:, t, 1:2], in_=tmp[:], axis=AX)
            # after slot reads: turn this tile's triu contribution into totals
            nc.tensor.matmul(out=csps[:], lhsT=trils_sb[:], rhs=oh[:],
                             start=False, stop=(t == NT - 1), skip_group_check=True)

            # dispatch: scatter gate-scaled bf16 token rows into xb
            xrow = xpool.tile([P, D], BF, tag="xrow")
            nc.sync.dma_start(xrow[:], xbf[ts(t, P), :])
            for k, g in ((0, g1), (1, g2)):
                xs = xpool.tile([P, D], BF, tag=f"xs{k}")
                nc.vector.tensor_scalar_mul(xs[:], xrow[:], g[:, 0:1])
                si = rpool.tile([P, 1], I32, tag=f"si{k}")
                nc.vector.tensor_copy(out=si[:], in_=slots_sb[:, t, k:k + 1])
                nc.gpsimd.indirect_dma_start(
                    out=xb[:], out_offset=IndirectOffsetOnAxis(ap=si[:, 0:1], axis=0),
                    in_=xs[:], in_offset=None,
                    bounds_check=bnd_reg, oob_is_err=False)

        # ================= P3: shared expert (resident output) ================
        for t in range(NT):
            xtb = xpool.tile([P, DJ, P], BF, tag="xtb")
            nc.sync.dma_start(xtb[:], xtbf_r[:, :, ts(t, P)])
            hT = hpool.tile([P, HJ, P], BF, tag="hT")
            ffn_tile(nc, xtb, w1s_sb, w3s_sb, w2s_sb, hT, ps_h, ps_ob,
                     ob_sb=S_sb[:, t, :])

        # ================= P2: expert FFN over xb ============================
        for e in range(E):
            w1sb = wpool.tile([P, DJ, H], BF, tag="w1")
            nc.sync.dma_start(w1sb[:], w1t[e].rearrange("(a p) h -> p a h", p=P))
            w3sb = wpool.tile([P, DJ, H], BF, tag="w3")
            nc.sync.dma_start(w3sb[:], w3t[e].rearrange("(a p) h -> p a h", p=P))
            w2sb = wpool.tile([P, HJ, D], BF, tag="w2")
            nc.sync.dma_start(w2sb[:], w2t[e].rearrange("(j p) d -> p j d", p=P))
            xT = xpool.tile([P, DJ, P], BF, tag="xbT")
            nc.sync.dma_start_transpose(xT[:], xb[ts(e, CAP), :])
            hT = hpool.tile([P, HJ, P], BF, tag="hT")
            ob_sb = obpool.tile([P, D], BF, tag="obrow")
            ffn_tile(nc, xT, w1sb, w3sb, w2sb, hT, ps_h, ps_ob, ob_sb=ob_sb)
            nc.sync.dma_start(ob[ts(e, CAP), :], ob_sb[:])

        # ================= P4: combine =======================================
        for t in range(NT):
            ga = []
            for k in range(2):
                si = cpool.tile([P, 1], I32, tag=f"ci{k}")
                nc.vector.tensor_copy(out=si[:], in_=slots_sb[:, t, k:k + 1])
                g = cpool.tile([P, D], BF, tag=f"g{k}")
                nc.gpsimd.indirect_dma_start(
                    out=g[:], out_offset=None,
                    in_=ob[:], in_offset=IndirectOffsetOnAxis(ap=si[:, 0:1], axis=0),
                    bounds_check=bnd_reg, oob_is_err=False)
                ga.append(g)
            of = cpool.tile([P, D], F32, tag="of")
            nc.vector.tensor_add(out=of[:], in0=ga[0][:], in1=ga[1][:])
            nc.vector.tensor_add(out=of[:], in0=of[:], in1=S_sb[:, t, :])
            nc.sync.dma_start(out[ts(t, P), :], of[:])

    nc.finalize()
    return nc


_cache = {}


def _prep_inputs(x, gate_w, w1, w2, w3, w1s, w2s, w3s, expert_bias):
    bf = ml_dtypes.bfloat16
    shared = {
        "gwt": np.ascontiguousarray(gate_w.T).astype(np.float32),
        "w1t": np.ascontiguousarray(w1.transpose(0, 2, 1)).astype(bf),
        "w3t": np.ascontiguousarray(w3.transpose(0, 2, 1)).astype(bf),
        "w2t": np.ascontiguousarray(w2.transpose(0, 2, 1)).astype(bf),
        "w1st": np.ascontiguousarray(w1s.T).astype(bf),
        "w3st": np.ascontiguousarray(w3s.T).astype(bf),
        "w2st": np.ascontiguousarray(w2s.T).astype(bf),
        "biasb": np.tile(expert_bias.astype(np.float32), (P, 1)),
        "iotab": np.tile(np.arange(E, dtype=np.float32), (P, 1)),
        "ebasem1": np.tile((np.arange(E) * CAP - 1).astype(np.float32), (P, 1)),
        "triu": np.triu(np.ones((P, P), dtype=np.float32)),
        "trils": np.tril(np.ones((P, P), dtype=np.float32), k=-1),
    }
    in_maps = []
    for j in range(NCORES):
        xs = x[j * TPC:(j + 1) * TPC]
        m = dict(shared)
        m["xt32"] = np.ascontiguousarray(xs.T).astype(np.float32)
        m["xbf"] = np.ascontiguousarray(xs).astype(bf)
        m["xtbf"] = np.ascontiguousarray(xs.T).astype(bf)
        in_maps.append(m)
    return in_maps


def kernel(x, gate_w, w1, w2, w3, w1s, w2s, w3s, expert_bias, _trace=False):
    x = np.asarray(x)
    in_maps = _prep_inputs(np.asarray(x, np.float32), np.asarray(gate_w),
                           np.asarray(w1), np.asarray(w2), np.asarray(w3),
                           np.asarray(w1s), np.asarray(w2s), np.asarray(w3s),
                           np.asarray(expert_bias))
    if "nc" not in _cache:
        _cache["nc"] = build_bass()
    res = bass_utils.run_bass_kernel_spmd(
        _cache["nc"], in_maps, core_ids=list(range(NCORES)), trace=_trace)
    out = np.concatenate([r["out"] for r in res.results], axis=0)
    _cache["last_results"] = res
    return out.astype(np.float32)


# revision 11
# speedup vs baseline: 43.6208x; 43.6208x over previous
"""MoE (top-2 routing, SwiGLU experts + shared expert) on 8 TRN2 NeuronCores.

Strategy: token-parallel across cores (2048 tokens/core), experts replicated.
Per core, entirely on device:
  P1 router: fp32 scores = sigmoid(x @ gate_w^T), top-2 via DVE max8/max_index,
     gate normalization, slot assignment via matmul-cumsum (triangular-ones
     matmuls) into a per-(core,expert) capacity buffer (128 slots/expert),
     dispatch = indirect row-scatter of gate-scaled bf16 token rows into xb.
  P2 expert FFN: for each of 64 experts, transpose-load its 128 xb rows,
     bf16 matmuls silu(x@w1^T)*(x@w3^T) @ w2^T -> ob rows (token-major).
  P3 shared expert: same FFN on natural token tiles, result resident in SBUF.
  P4 combine: indirect row-gather of each token's two expert output rows,
     out = gathered1 + gathered2 + shared.
No collectives; host only slices/casts/concatenates.
"""

import numpy as np
import ml_dtypes
from contextlib import ExitStack

import concourse.bass as bass
from concourse import bacc
import concourse.mybir as mybir
import concourse.tile as tile
from concourse.bass import ts, ds, IndirectOffsetOnAxis
from concourse import bass_utils

P = 128
NCORES = 8
N, D, H, E = 16384, 1024, 512, 64
TPC = N // NCORES        # 2048 tokens per core
NT = TPC // P            # 16 token tiles per core
DJ = D // P              # 8 contraction chunks over D
HJ = H // P              # 4 chunks over H
CAP = 128                # per-core per-expert slot capacity (1 tile)
NSLOT = E * CAP          # 8192
BIG = 1.0e7
SIM_SILU = False

BF = mybir.dt.bfloat16
F32 = mybir.dt.float32
I32 = mybir.dt.int32
U32 = mybir.dt.uint32
AX = mybir.AxisListType.X
OP = mybir.AluOpType
ACTF = mybir.ActivationFunctionType


def ffn_tile(nc, xT, w1sb, w3sb, w2sb, hT, ps_h, ps_ob, ob_dst_f32=None,
             ob_sb=None):
    """SwiGLU FFN for one 128-token tile.

    xT:   [P, DJ, P] bf16 (D on partitions, tokens on free)
    w1sb/w3sb: [P, DJ, H] bf16 (lhsT blocks, d on partitions, h on free)
    w2sb: [P, HJ, D] bf16 (h on partitions, d on free)
    hT:   [P, HJ, P] bf16 scratch tile (h on partitions, tokens free)
    writes token-major [P, D] output into ob_sb (bf16 tile slice).
    """
    for j in range(HJ):
        h1 = ps_h.tile([P, P], F32, tag="h1")
        h3 = ps_h.tile([P, P], F32, tag="h3")
        for i in range(DJ):
            nc.tensor.matmul(out=h1[:], lhsT=w1sb[:, i, ts(j, P)], rhs=xT[:, i, :],
                             start=(i == 0), stop=(i == DJ - 1))
        for i in range(DJ):
            nc.tensor.matmul(out=h3[:], lhsT=w3sb[:, i, ts(j, P)], rhs=xT[:, i, :],
                             start=(i == 0), stop=(i == DJ - 1))
        s1 = ps_h.pool_sb.tile([P, P], F32, tag="silu")
        if SIM_SILU:  # CoreSim has no Silu; emulate via sigmoid * x
            nc.scalar.activation(s1[:], h1[:], ACTF.Sigmoid)
            nc.vector.tensor_mul(out=s1[:], in0=s1[:], in1=h1[:])
        else:
            nc.scalar.activation(s1[:], h1[:], ACTF.Silu)
        nc.vector.tensor_mul(out=hT[:, j, :], in0=s1[:], in1=h3[:])
    for nh in range(2):
        obps = ps_ob.tile([P, D // 2], F32, tag="ob")
        for j in range(HJ):
            nc.tensor.matmul(out=obps[:], lhsT=hT[:, j, :],
                             rhs=w2sb[:, j, ds(nh * (D // 2), D // 2)],
                             start=(j == 0), stop=(j == HJ - 1))
        nc.vector.tensor_copy(out=ob_sb[:, ds(nh * (D // 2), D // 2)], in_=obps[:])


def build_bass():
    nc = bacc.Bacc("TRN2", target_bir_lowering=False)
    # ---- I/O ----
    xt32 = nc.dram_tensor("xt32", [NT, P, DJ, P], F32, kind="ExternalInput")
    xbf = nc.dram_tensor("xbf", [TPC, D], BF, kind="ExternalInput")
    xtbf = nc.dram_tensor("xtbf", [NT, P, DJ, P], BF, kind="ExternalInput")
    gwt = nc.dram_tensor("gwt", [P, DJ, E], F32, kind="ExternalInput")
    w1t = nc.dram_tensor("w1t", [E, P, DJ, H], BF, kind="ExternalInput")
    w3t = nc.dram_tensor("w3t", [E, P, DJ, H], BF, kind="ExternalInput")
    w2t = nc.dram_tensor("w2t", [E, P, HJ, D], BF, kind="ExternalInput")
    w1st = nc.dram_tensor("w1st", [P, DJ, H], BF, kind="ExternalInput")
    w3st = nc.dram_tensor("w3st", [P, DJ, H], BF, kind="ExternalInput")
    w2st = nc.dram_tensor("w2st", [P, HJ, D], BF, kind="ExternalInput")
    biasb = nc.dram_tensor("biasb", [P, E], F32, kind="ExternalInput")
    iotab = nc.dram_tensor("iotab", [P, E], F32, kind="ExternalInput")
    ebasem1 = nc.dram_tensor("ebasem1", [P, E], F32, kind="ExternalInput")
    triu = nc.dram_tensor("triu", [P, P], F32, kind="ExternalInput")
    trils = nc.dram_tensor("trils", [P, P], F32, kind="ExternalInput")
    out = nc.dram_tensor("out", [TPC, D], F32, kind="ExternalOutput")
    xb = nc.dram_tensor("xb", [NSLOT, D], BF, kind="Internal")
    ob = nc.dram_tensor("ob", [NSLOT, D], BF, kind="Internal")


    with ExitStack() as ctx:
        tc = ctx.enter_context(tile.TileContext(nc))
        const = ctx.enter_context(tc.tile_pool(name="const", bufs=1))
        swpool = ctx.enter_context(tc.tile_pool(name="sw", bufs=1))
        spool = ctx.enter_context(tc.tile_pool(name="sres", bufs=1))
        wpool = ctx.enter_context(tc.tile_pool(name="wstream", bufs=2))
        xpool = ctx.enter_context(tc.tile_pool(name="xtiles", bufs=2))
        rpool = ctx.enter_context(tc.tile_pool(name="router", bufs=2))
        hpool = ctx.enter_context(tc.tile_pool(name="hsb", bufs=3))
        obpool = ctx.enter_context(tc.tile_pool(name="obsb", bufs=3))
        cpool = ctx.enter_context(tc.tile_pool(name="combine", bufs=2))
        ps_r = ctx.enter_context(tc.tile_pool(name="ps_r", bufs=1, space="PSUM"))
        ps_cs = ctx.enter_context(tc.tile_pool(name="ps_cs", bufs=1, space="PSUM"))
        ps_h = ctx.enter_context(tc.tile_pool(name="ps_h", bufs=2, space="PSUM"))
        ps_ob = ctx.enter_context(tc.tile_pool(name="ps_ob", bufs=2, space="PSUM"))
        ps_h.pool_sb = hpool  # convenience for ffn_tile silu scratch

        # ---- consts & resident tensors ----
        gw_sb = const.tile([P, DJ, E], F32)
        nc.sync.dma_start(gw_sb[:], gwt[:])
        bias_sb = const.tile([P, E], F32)
        nc.sync.dma_start(bias_sb[:], biasb[:])
        iota_sb = const.tile([P, E], F32)
        nc.sync.dma_start(iota_sb[:], iotab[:])
        ebase_sb = const.tile([P, E], F32)
        nc.sync.dma_start(ebase_sb[:], ebasem1[:])
        triu_sb = const.tile([P, P], F32)
        nc.sync.dma_start(triu_sb[:], triu[:])
        trils_sb = const.tile([P, P], F32)
        nc.sync.dma_start(trils_sb[:], trils[:])

        w1s_sb = swpool.tile([P, DJ, H], BF)
        nc.sync.dma_start(w1s_sb[:], w1st[:])
        w3s_sb = swpool.tile([P, DJ, H], BF)
        nc.sync.dma_start(w3s_sb[:], w3st[:])
        w2s_sb = swpool.tile([P, HJ, D], BF)
        nc.sync.dma_start(w2s_sb[:], w2st[:])

        S_sb = spool.tile([P, NT, D], BF)          # shared-expert outputs, resident
        slots_sb = spool.tile([P, NT, 2], F32)     # slot ids per token per pick

        bnd_reg = nc.gpsimd.alloc_register("bnd")
        nc.gpsimd.reg_mov(bnd_reg, NSLOT - 1)

        # ---- zero-fill xb (pad rows must be finite) ----
        zeros_sb = const.tile([P, 4, D], BF)
        nc.vector.memset(zeros_sb[:], 0.0)
        for c in range(NSLOT // 512):
            nc.gpsimd.dma_start(
                xb[ts(c, 512), :].rearrange("(p q) d -> p q d", p=P),
                zeros_sb[:],
            )

        # ================= P1: router + slot assignment + dispatch ============
        csps = ps_cs.tile([P, E], F32)  # running cumsum psum, persists across tiles
        for t in range(NT):
            xt_sb = rpool.tile([P, DJ, P], F32, tag="xt32")
            nc.sync.dma_start(xt_sb[:], xt32[t])
            scps = ps_r.tile([P, E], F32, tag="scores")
            for i in range(DJ):
                nc.tensor.matmul(out=scps[:], lhsT=xt_sb[:, i, :], rhs=gw_sb[:, i, :],
                                 start=(i == 0), stop=(i == DJ - 1))
            scores = rpool.tile([P, E], F32, tag="scores_sb")
            nc.scalar.activation(scores[:], scps[:], ACTF.Sigmoid)
            sel = rpool.tile([P, E], F32, tag="sel")
            nc.vector.tensor_add(out=sel[:], in0=scores[:], in1=bias_sb[:])
            mx = rpool.tile([P, 8], F32, tag="mx")
            nc.vector.max(out=mx[:], in_=sel[:])
            mxi = rpool.tile([P, 8], U32, tag="mxi")
            nc.vector.max_index(out=mxi[:], in_max=mx[:], in_values=sel[:])
            idxf = rpool.tile([P, 2], F32, tag="idxf")
            nc.vector.tensor_copy(out=idxf[:], in_=mxi[:, 0:2])
            oh1 = rpool.tile([P, E], F32, tag="oh1")
            nc.vector.tensor_scalar(oh1[:], iota_sb[:], idxf[:, 0:1], None,
                                    op0=OP.is_equal)
            oh2 = rpool.tile([P, E], F32, tag="oh2")
            nc.vector.tensor_scalar(oh2[:], iota_sb[:], idxf[:, 1:2], None,
                                    op0=OP.is_equal)
            # raw scores at the two picks; normalized gates
            tmp = rpool.tile([P, E], F32, tag="tmp")
            nc.vector.tensor_mul(out=tmp[:], in0=scores[:], in1=oh1[:])
            val1 = rpool.tile([P, 1], F32, tag="val1")
            nc.vector.reduce_sum(out=val1[:], in_=tmp[:], axis=AX)
            nc.vector.tensor_mul(out=tmp[:], in0=scores[:], in1=oh2[:])
            val2 = rpool.tile([P, 1], F32, tag="val2")
            nc.vector.reduce_sum(out=val2[:], in_=tmp[:], axis=AX)
            den = rpool.tile([P, 1], F32, tag="den")
            nc.vector.tensor_add(out=den[:], in0=val1[:], in1=val2[:])
            nc.vector.tensor_scalar_add(den[:], den[:], 1e-20)
            rec = rpool.tile([P, 1], F32, tag="rec")
            nc.vector.reciprocal(rec[:], den[:])
            g1 = rpool.tile([P, 1], F32, tag="g1")
            nc.vector.tensor_mul(out=g1[:], in0=val1[:], in1=rec[:])
            g2 = rpool.tile([P, 1], F32, tag="g2")
            nc.vector.tensor_mul(out=g2[:], in0=val2[:], in1=rec[:])

            # cumulative per-expert rank (inclusive), then convert the psum to
            # column totals for the next tile by adding strictly-lower part.
            oh = rpool.tile([P, E], F32, tag="ohsum")
            nc.vector.tensor_add(out=oh[:], in0=oh1[:], in1=oh2[:])
            nc.tensor.matmul(out=csps[:], lhsT=triu_sb[:], rhs=oh[:],
                             start=(t == 0), stop=False, skip_group_check=True)
            # slot = e*CAP + (incl-1) if incl <= CAP else BIG
            valid = rpool.tile([P, E], F32, tag="valid")
            nc.vector.tensor_scalar(valid[:], csps[:], float(CAP), None, op0=OP.is_le)
            slotm = rpool.tile([P, E], F32, tag="slotm")
            nc.vector.tensor_add(out=slotm[:], in0=csps[:], in1=ebase_sb[:])
            nc.vector.tensor_scalar_add(slotm[:], slotm[:], -BIG)
            nc.vector.tensor_mul(out=slotm[:], in0=slotm[:], in1=valid[:])
            nc.vector.tensor_scalar_add(slotm[:], slotm[:], BIG)
            nc.vector.tensor_mul(out=tmp[:], in0=slotm[:], in1=oh1[:])
            nc.vector.reduce_sum(out=slots_sb[:, t, 0:1], in_=tmp[:], axis=AX)
            nc.vector.tensor_mul(out=tmp[:], in0=slotm[:], in1=oh2[:])
            nc.vector.reduce_sum(out=slots_sb[:, t, 1:2], in_=tmp[:], axis=AX)
            # after slot reads: turn this tile's triu contribution into totals
            nc.tensor.matmul(out=csps[:], lhsT=trils_sb[:], rhs=oh[:],
                             start=False, stop=(t == NT - 1), skip_group_check=True)

            # dispatch: scatter gate-scaled bf16 token rows into xb
            xrow = xpool.tile([P, D], BF, tag="xrow")
            nc.sync.dma_start(xrow[:], xbf[ts(t, P), :])
            for k, g in ((0, g1), (1, g2)):
                xs = xpool.tile([P, D], BF, tag=f"xs{k}")
                nc.vector.tensor_scalar_mul(xs[:], xrow[:], g[:, 0:1])
                si = rpool.tile([P, 1], I32, tag=f"si{k}")
                nc.vector.tensor_copy(out=si[:], in_=slots_sb[:, t, k:k + 1])
                nc.gpsimd.indirect_dma_start(
                    out=xb[:], out_offset=IndirectOffsetOnAxis(ap=si[:, 0:1], axis=0),
                    in_=xs[:], in_offset=None,
                    bounds_check=bnd_reg, oob_is_err=False)

        # ================= P3: shared expert (resident output) ================
        for t in range(NT):
            xtb = xpool.tile([P, DJ, P], BF, tag="xtb")
            nc.sync.dma_start(xtb[:], xtbf[t])
            hT = hpool.tile([P, HJ, P], BF, tag="hT")
            ffn_tile(nc, xtb, w1s_sb, w3s_sb, w2s_sb, hT, ps_h, ps_ob,
                     ob_sb=S_sb[:, t, :])

        # ================= P2: expert FFN over xb ============================
        for e in range(E):
            w1sb = wpool.tile([P, DJ, H], BF, tag="w1")
            nc.sync.dma_start(w1sb[:], w1t[e])
            w3sb = wpool.tile([P, DJ, H], BF, tag="w3")
            nc.sync.dma_start(w3sb[:], w3t[e])
            w2sb = wpool.tile([P, HJ, D], BF, tag="w2")
            nc.sync.dma_start(w2sb[:], w2t[e])
            xT = xpool.tile([P, DJ, P], BF, tag="xbT")
            nc.sync.dma_start_transpose(xT[:], xb[ts(e, CAP), :])
            hT = hpool.tile([P, HJ, P], BF, tag="hT")
            ob_sb = obpool.tile([P, D], BF, tag="obrow")
            ffn_tile(nc, xT, w1sb, w3sb, w2sb, hT, ps_h, ps_ob, ob_sb=ob_sb)
            nc.sync.dma_start(ob[ts(e, CAP), :], ob_sb[:])

        # ================= P4: combine =======================================
        for t in range(NT):
            ga = []
            for k in range(2):
                si = cpool.tile([P, 1], I32, tag=f"ci{k}")
                nc.vector.tensor_copy(out=si[:], in_=slots_sb[:, t, k:k + 1])
                g = cpool.tile([P, D], BF, tag=f"g{k}")
                nc.gpsimd.indirect_dma_start(
                    out=g[:], out_offset=None,
                    in_=ob[:], in_offset=IndirectOffsetOnAxis(ap=si[:, 0:1], axis=0),
                    bounds_check=bnd_reg, oob_is_err=False)
                ga.append(g)
            of = cpool.tile([P, D], F32, tag="of")
            nc.vector.tensor_add(out=of[:], in0=ga[0][:], in1=ga[1][:])
            nc.vector.tensor_add(out=of[:], in0=of[:], in1=S_sb[:, t, :])
            nc.sync.dma_start(out[ts(t, P), :], of[:])

    nc.finalize()
    return nc


_cache = {}


def _prep_inputs(x, gate_w, w1, w2, w3, w1s, w2s, w3s, expert_bias):
    bf = ml_dtypes.bfloat16
    def swz_dh(wt):   # [D, H] -> [P, DJ, H] partition-major
        return np.ascontiguousarray(wt.reshape(DJ, P, wt.shape[-1]).transpose(1, 0, 2))

    def swz_hd(wt):   # [H, D] -> [P, HJ, D]
        return np.ascontiguousarray(wt.reshape(HJ, P, wt.shape[-1]).transpose(1, 0, 2))

    shared = {
        "gwt": swz_dh(np.ascontiguousarray(gate_w.T)).astype(np.float32),
        "w1t": np.stack([swz_dh(w1[e].T) for e in range(E)]).astype(bf),
        "w3t": np.stack([swz_dh(w3[e].T) for e in range(E)]).astype(bf),
        "w2t": np.stack([swz_hd(w2[e].T) for e in range(E)]).astype(bf),
        "w1st": swz_dh(w1s.T).astype(bf),
        "w3st": swz_dh(w3s.T).astype(bf),
        "w2st": swz_hd(w2s.T).astype(bf),
        "biasb": np.tile(expert_bias.astype(np.float32), (P, 1)),
        "iotab": np.tile(np.arange(E, dtype=np.float32), (P, 1)),
        "ebasem1": np.tile((np.arange(E) * CAP - 1).astype(np.float32), (P, 1)),
        "triu": np.triu(np.ones((P, P), dtype=np.float32)),
        "trils": np.tril(np.ones((P, P), dtype=np.float32), k=-1),
    }
    in_maps = []
    for j in range(NCORES):
        xs = x[j * TPC:(j + 1) * TPC]
        m = dict(shared)
        xsw = np.ascontiguousarray(
            xs.reshape(NT, P, DJ, P).transpose(0, 3, 2, 1))
        m["xt32"] = xsw.astype(np.float32)
        m["xbf"] = np.ascontiguousarray(xs).astype(bf)
        m["xtbf"] = xsw.astype(bf)
        in_maps.append(m)
    return in_maps


def kernel(x, gate_w, w1, w2, w3, w1s, w2s, w3s, expert_bias, _trace=False):
    x = np.asarray(x)
    in_maps = _prep_inputs(np.asarray(x, np.float32), np.asarray(gate_w),
                           np.asarray(w1), np.asarray(w2), np.asarray(w3),
                           np.asarray(w1s), np.asarray(w2s), np.asarray(w3s),
                           np.asarray(expert_bias))
    if "nc" not in _cache:
        _cache["nc"] = build_bass()
    res = bass_utils.run_bass_kernel_spmd(
        _cache["nc"], in_maps, core_ids=list(range(NCORES)), trace=_trace)
    out = np.concatenate([r["out"] for r in res.results], axis=0)
    _cache["last_results"] = res
    return out.astype(np.float32)


# revision 14
# speedup vs baseline: 48.0475x; 1.1015x over previous
"""MoE (top-2 routing, SwiGLU experts + shared expert) on 8 TRN2 NeuronCores.

Strategy: token-parallel across cores (2048 tokens/core), experts replicated.
Per core, entirely on device:
  P1 router: fp32 scores = sigmoid(x @ gate_w^T), top-2 via DVE max8/max_index,
     gate normalization, slot assignment via matmul-cumsum (triangular-ones
     matmuls) into a per-(core,expert) capacity buffer (128 slots/expert),
     dispatch = indirect row-scatter of gate-scaled bf16 token rows into xb.
  P2 expert FFN: for each of 64 experts, transpose-load its 128 xb rows,
     bf16 matmuls silu(x@w1^T)*(x@w3^T) @ w2^T -> ob rows (token-major).
  P3 shared expert: same FFN on natural token tiles, result resident in SBUF.
  P4 combine: indirect row-gather of each token's two expert output rows,
     out = gathered1 + gathered2 + shared.
No collectives; host only slices/casts/concatenates.
"""

import numpy as np
import ml_dtypes
from contextlib import ExitStack

import concourse.bass as bass
from concourse import bacc
import concourse.mybir as mybir
import concourse.tile as tile
from concourse.bass import ts, ds, IndirectOffsetOnAxis
from concourse import bass_utils

P = 128
NCORES = 8
N, D, H, E = 16384, 1024, 512, 64
TPC = N // NCORES        # 2048 tokens per core
NT = TPC // P            # 16 token tiles per core
DJ = D // P              # 8 contraction chunks over D
HJ = H // P              # 4 chunks over H
CAP = 128                # per-core per-expert slot capacity (1 tile)
NSLOT = E * CAP          # 8192
BIG = 1.0e7
SIM_SILU = False
PHASES = (1, 2, 3, 4)

BF = mybir.dt.bfloat16
F32 = mybir.dt.float32
I32 = mybir.dt.int32
U32 = mybir.dt.uint32
AX = mybir.AxisListType.X
OP = mybir.AluOpType
ACTF = mybir.ActivationFunctionType


def ffn_tile(nc, xT, w1sb, w3sb, w2sb, hT, ps_h, ps_ob, ob_dst_f32=None,
             ob_sb=None):
    """SwiGLU FFN for one 128-token tile.

    xT:   [P, DJ, P] bf16 (D on partitions, tokens on free)
    w1sb/w3sb: [P, DJ, H] bf16 (lhsT blocks, d on partitions, h on free)
    w2sb: [P, HJ, D] bf16 (h on partitions, d on free)
    hT:   [P, HJ, P] bf16 scratch tile (h on partitions, tokens free)
    writes token-major [P, D] output into ob_sb (bf16 tile slice).
    """
    for j in range(HJ):
        h1 = ps_h.tile([P, P], F32, tag="h1")
        h3 = ps_h.tile([P, P], F32, tag="h3")
        for i in range(DJ):
            nc.tensor.matmul(out=h1[:], lhsT=w1sb[:, i, ts(j, P)], rhs=xT[:, i, :],
                             start=(i == 0), stop=(i == DJ - 1))
        for i in range(DJ):
            nc.tensor.matmul(out=h3[:], lhsT=w3sb[:, i, ts(j, P)], rhs=xT[:, i, :],
                             start=(i == 0), stop=(i == DJ - 1))
        s1 = ps_h.pool_sb.tile([P, P], F32, tag="silu")
        if SIM_SILU:  # CoreSim has no Silu; emulate via sigmoid * x
            nc.scalar.activation(s1[:], h1[:], ACTF.Sigmoid)
            nc.vector.tensor_mul(out=s1[:], in0=s1[:], in1=h1[:])
        else:
            nc.scalar.activation(s1[:], h1[:], ACTF.Silu)
        nc.vector.tensor_mul(out=hT[:, j, :], in0=s1[:], in1=h3[:])
    for nh in range(2):
        obps = ps_ob.tile([P, D // 2], F32, tag="ob")
        for j in range(HJ):
            nc.tensor.matmul(out=obps[:], lhsT=hT[:, j, :],
                             rhs=w2sb[:, j, ds(nh * (D // 2), D // 2)],
                             start=(j == 0), stop=(j == HJ - 1))
        nc.vector.tensor_copy(out=ob_sb[:, ds(nh * (D // 2), D // 2)], in_=obps[:])


def build_bass():
    nc = bacc.Bacc("TRN2", target_bir_lowering=False)
    # ---- I/O ----
    xt32 = nc.dram_tensor("xt32", [NT, P, DJ, P], F32, kind="ExternalInput")
    xbf = nc.dram_tensor("xbf", [TPC, D], BF, kind="ExternalInput")
    xtbf = nc.dram_tensor("xtbf", [NT, P, DJ, P], BF, kind="ExternalInput")
    gwt = nc.dram_tensor("gwt", [P, DJ, E], F32, kind="ExternalInput")
    w1t = nc.dram_tensor("w1t", [E, P, DJ, H], BF, kind="ExternalInput")
    w3t = nc.dram_tensor("w3t", [E, P, DJ, H], BF, kind="ExternalInput")
    w2t = nc.dram_tensor("w2t", [E, P, HJ, D], BF, kind="ExternalInput")
    w1st = nc.dram_tensor("w1st", [P, DJ, H], BF, kind="ExternalInput")
    w3st = nc.dram_tensor("w3st", [P, DJ, H], BF, kind="ExternalInput")
    w2st = nc.dram_tensor("w2st", [P, HJ, D], BF, kind="ExternalInput")
    biasb = nc.dram_tensor("biasb", [P, E], F32, kind="ExternalInput")
    iotab = nc.dram_tensor("iotab", [P, E], F32, kind="ExternalInput")
    ebasem1 = nc.dram_tensor("ebasem1", [P, E], F32, kind="ExternalInput")
    triu = nc.dram_tensor("triu", [P, P], F32, kind="ExternalInput")
    trils = nc.dram_tensor("trils", [P, P], F32, kind="ExternalInput")
    out = nc.dram_tensor("out", [TPC, D], F32, kind="ExternalOutput")
    xb = nc.dram_tensor("xb", [NSLOT, D], BF, kind="Internal")
    sh_hbm = nc.dram_tensor("sh_hbm", [TPC, D], BF, kind="Internal")
    ob = nc.dram_tensor("ob", [NSLOT, D], BF, kind="Internal")


    with ExitStack() as ctx:
        tc = ctx.enter_context(tile.TileContext(nc))
        const = ctx.enter_context(tc.tile_pool(name="const", bufs=1))
        swpool = ctx.enter_context(tc.tile_pool(name="sw", bufs=1))
        spool = ctx.enter_context(tc.tile_pool(name="sres", bufs=1))
        wpool = ctx.enter_context(tc.tile_pool(name="wstream", bufs=3))
        xpool = ctx.enter_context(tc.tile_pool(name="xtiles", bufs=2))
        rpool = ctx.enter_context(tc.tile_pool(name="router", bufs=2))
        hpool = ctx.enter_context(tc.tile_pool(name="hsb", bufs=3))
        obpool = ctx.enter_context(tc.tile_pool(name="obsb", bufs=3))
        cpool = ctx.enter_context(tc.tile_pool(name="combine", bufs=2))
        ps_r = ctx.enter_context(tc.tile_pool(name="ps_r", bufs=1, space="PSUM"))
        ps_cs = ctx.enter_context(tc.tile_pool(name="ps_cs", bufs=1, space="PSUM"))
        ps_h = ctx.enter_context(tc.tile_pool(name="ps_h", bufs=2, space="PSUM"))
        ps_ob = ctx.enter_context(tc.tile_pool(name="ps_ob", bufs=2, space="PSUM"))
        ps_h.pool_sb = hpool  # convenience for ffn_tile silu scratch

        # ---- consts & resident tensors ----
        gw_sb = const.tile([P, DJ, E], F32)
        nc.sync.dma_start(gw_sb[:], gwt[:])
        bias_sb = const.tile([P, E], F32)
        nc.sync.dma_start(bias_sb[:], biasb[:])
        iota_sb = const.tile([P, E], F32)
        nc.sync.dma_start(iota_sb[:], iotab[:])
        ebase_sb = const.tile([P, E], F32)
        nc.sync.dma_start(ebase_sb[:], ebasem1[:])
        triu_sb = const.tile([P, P], F32)
        nc.sync.dma_start(triu_sb[:], triu[:])
        trils_sb = const.tile([P, P], F32)
        nc.sync.dma_start(trils_sb[:], trils[:])

        w1s_sb = swpool.tile([P, DJ, H], BF)
        nc.sync.dma_start(w1s_sb[:], w1st[:])
        w3s_sb = swpool.tile([P, DJ, H], BF)
        nc.sync.dma_start(w3s_sb[:], w3st[:])
        w2s_sb = swpool.tile([P, HJ, D], BF)
        nc.sync.dma_start(w2s_sb[:], w2st[:])

        slots_sb = spool.tile([P, NT, 2], F32)     # slot ids per token per pick

        bnd_reg = nc.gpsimd.alloc_register("bnd")
        nc.gpsimd.reg_mov(bnd_reg, NSLOT - 1)

        # ---- zero-fill xb (pad rows must be finite) ----
        zeros_sb = const.tile([P, 4, D], BF)
        nc.vector.memset(zeros_sb[:], 0.0)
        for c in range(NSLOT // 512):
            nc.gpsimd.dma_start(
                xb[ts(c, 512), :].rearrange("(p q) d -> p q d", p=P),
                zeros_sb[:],
            )

        # ================= P1: router + slot assignment + dispatch ============
        P1on = 1 in PHASES
        csps = ps_cs.tile([P, E], F32)  # running cumsum psum, persists across tiles
        for t in range(NT) if P1on else []:
            xt_sb = rpool.tile([P, DJ, P], F32, tag="xt32")
            nc.sync.dma_start(xt_sb[:], xt32[t])
            scps = ps_r.tile([P, E], F32, tag="scores")
            for i in range(DJ):
                nc.tensor.matmul(out=scps[:], lhsT=xt_sb[:, i, :], rhs=gw_sb[:, i, :],
                                 start=(i == 0), stop=(i == DJ - 1))
            scores = rpool.tile([P, E], F32, tag="scores_sb")
            nc.scalar.activation(scores[:], scps[:], ACTF.Sigmoid)
            sel = rpool.tile([P, E], F32, tag="sel")
            nc.vector.tensor_add(out=sel[:], in0=scores[:], in1=bias_sb[:])
            mx = rpool.tile([P, 8], F32, tag="mx")
            nc.vector.max(out=mx[:], in_=sel[:])
            mxi = rpool.tile([P, 8], U32, tag="mxi")
            nc.vector.max_index(out=mxi[:], in_max=mx[:], in_values=sel[:])
            idxf = rpool.tile([P, 2], F32, tag="idxf")
            nc.vector.tensor_copy(out=idxf[:], in_=mxi[:, 0:2])
            oh1 = rpool.tile([P, E], F32, tag="oh1")
            nc.vector.tensor_scalar(oh1[:], iota_sb[:], idxf[:, 0:1], None,
                                    op0=OP.is_equal)
            oh2 = rpool.tile([P, E], F32, tag="oh2")
            nc.vector.tensor_scalar(oh2[:], iota_sb[:], idxf[:, 1:2], None,
                                    op0=OP.is_equal)
            # raw scores at the two picks; normalized gates
            tmp = rpool.tile([P, E], F32, tag="tmp")
            nc.vector.tensor_mul(out=tmp[:], in0=scores[:], in1=oh1[:])
            val1 = rpool.tile([P, 1], F32, tag="val1")
            nc.vector.reduce_sum(out=val1[:], in_=tmp[:], axis=AX)
            nc.vector.tensor_mul(out=tmp[:], in0=scores[:], in1=oh2[:])
            val2 = rpool.tile([P, 1], F32, tag="val2")
            nc.vector.reduce_sum(out=val2[:], in_=tmp[:], axis=AX)
            den = rpool.tile([P, 1], F32, tag="den")
            nc.vector.tensor_add(out=den[:], in0=val1[:], in1=val2[:])
            nc.vector.tensor_scalar_add(den[:], den[:], 1e-20)
            rec = rpool.tile([P, 1], F32, tag="rec")
            nc.vector.reciprocal(rec[:], den[:])
            g1 = rpool.tile([P, 1], F32, tag="g1")
            nc.vector.tensor_mul(out=g1[:], in0=val1[:], in1=rec[:])
            g2 = rpool.tile([P, 1], F32, tag="g2")
            nc.vector.tensor_mul(out=g2[:], in0=val2[:], in1=rec[:])

            # cumulative per-expert rank (inclusive), then convert the psum to
            # column totals for the next tile by adding strictly-lower part.
            oh = rpool.tile([P, E], F32, tag="ohsum")
            nc.vector.tensor_add(out=oh[:], in0=oh1[:], in1=oh2[:])
            nc.tensor.matmul(out=csps[:], lhsT=triu_sb[:], rhs=oh[:],
                             start=(t == 0), stop=False, skip_group_check=True)
            # slot = e*CAP + (incl-1) if incl <= CAP else BIG
            valid = rpool.tile([P, E], F32, tag="valid")
            nc.vector.tensor_scalar(valid[:], csps[:], float(CAP), None, op0=OP.is_le)
            slotm = rpool.tile([P, E], F32, tag="slotm")
            nc.vector.tensor_add(out=slotm[:], in0=csps[:], in1=ebase_sb[:])
            nc.vector.tensor_scalar_add(slotm[:], slotm[:], -BIG)
            nc.vector.tensor_mul(out=slotm[:], in0=slotm[:], in1=valid[:])
            nc.vector.tensor_scalar_add(slotm[:], slotm[:], BIG)
            nc.vector.tensor_mul(out=tmp[:], in0=slotm[:], in1=oh1[:])
            nc.vector.reduce_sum(out=slots_sb[:, t, 0:1], in_=tmp[:], axis=AX)
            nc.vector.tensor_mul(out=tmp[:], in0=slotm[:], in1=oh2[:])
            nc.vector.reduce_sum(out=slots_sb[:, t, 1:2], in_=tmp[:], axis=AX)
            # after slot reads: turn this tile's triu contribution into totals
            nc.tensor.matmul(out=csps[:], lhsT=trils_sb[:], rhs=oh[:],
                             start=False, stop=(t == NT - 1), skip_group_check=True)

            # dispatch: scatter gate-scaled bf16 token rows into xb
            xrow = xpool.tile([P, D], BF, tag="xrow")
            nc.sync.dma_start(xrow[:], xbf[ts(t, P), :])
            for k, g in ((0, g1), (1, g2)):
                xs = xpool.tile([P, D], BF, tag=f"xs{k}")
                nc.vector.tensor_scalar_mul(xs[:], xrow[:], g[:, 0:1])
                si = rpool.tile([P, 1], I32, tag=f"si{k}")
                nc.vector.tensor_copy(out=si[:], in_=slots_sb[:, t, k:k + 1])
                nc.gpsimd.indirect_dma_start(
                    out=xb[:], out_offset=IndirectOffsetOnAxis(ap=si[:, 0:1], axis=0),
                    in_=xs[:], in_offset=None,
                    bounds_check=bnd_reg, oob_is_err=False)

        # ================= P3: shared expert (resident output) ================
        for t in range(NT) if 3 in PHASES else []:
            xtb = xpool.tile([P, DJ, P], BF, tag="xtb")
            nc.sync.dma_start(xtb[:], xtbf[t])
            hT = hpool.tile([P, HJ, P], BF, tag="hT")
            s_sb = obpool.tile([P, D], BF, tag="obrow")
            ffn_tile(nc, xtb, w1s_sb, w3s_sb, w2s_sb, hT, ps_h, ps_ob,
                     ob_sb=s_sb)
            nc.sync.dma_start(sh_hbm[ts(t, P), :], s_sb[:])

        # ================= P2: expert FFN over xb ============================
        for e in range(E) if 2 in PHASES else []:
            w1sb = wpool.tile([P, DJ, H], BF, tag="w1")
            nc.sync.dma_start(w1sb[:], w1t[e])
            w3sb = wpool.tile([P, DJ, H], BF, tag="w3")
            nc.sync.dma_start(w3sb[:], w3t[e])
            w2sb = wpool.tile([P, HJ, D], BF, tag="w2")
            nc.sync.dma_start(w2sb[:], w2t[e])
            xT = xpool.tile([P, DJ, P], BF, tag="xbT")
            nc.sync.dma_start_transpose(xT[:], xb[ts(e, CAP), :])
            hT = hpool.tile([P, HJ, P], BF, tag="hT")
            ob_sb = obpool.tile([P, D], BF, tag="obrow")
            ffn_tile(nc, xT, w1sb, w3sb, w2sb, hT, ps_h, ps_ob, ob_sb=ob_sb)
            nc.sync.dma_start(ob[ts(e, CAP), :], ob_sb[:])

        # ================= P4: combine =======================================
        for t in range(NT) if 4 in PHASES else []:
            ga = []
            for k in range(2):
                si = cpool.tile([P, 1], I32, tag=f"ci{k}")
                nc.vector.tensor_copy(out=si[:], in_=slots_sb[:, t, k:k + 1])
                g = cpool.tile([P, D], BF, tag=f"g{k}")
                nc.gpsimd.indirect_dma_start(
                    out=g[:], out_offset=None,
                    in_=ob[:], in_offset=IndirectOffsetOnAxis(ap=si[:, 0:1], axis=0),
                    bounds_check=bnd_reg, oob_is_err=False)
                ga.append(g)
            s_t = cpool.tile([P, D], BF, tag="sht")
            nc.sync.dma_start(s_t[:], sh_hbm[ts(t, P), :])
            of = cpool.tile([P, D], F32, tag="of")
            nc.vector.tensor_add(out=of[:], in0=ga[0][:], in1=ga[1][:])
            nc.vector.tensor_add(out=of[:], in0=of[:], in1=s_t[:])
            nc.sync.dma_start(out[ts(t, P), :], of[:])

    nc.finalize()
    return nc


_cache = {}


def _prep_inputs(x, gate_w, w1, w2, w3, w1s, w2s, w3s, expert_bias):
    bf = ml_dtypes.bfloat16
    def swz_dh(wt):   # [D, H] -> [P, DJ, H] partition-major
        return np.ascontiguousarray(wt.reshape(DJ, P, wt.shape[-1]).transpose(1, 0, 2))

    def swz_hd(wt):   # [H, D] -> [P, HJ, D]
        return np.ascontiguousarray(wt.reshape(HJ, P, wt.shape[-1]).transpose(1, 0, 2))

    shared = {
        "gwt": swz_dh(np.ascontiguousarray(gate_w.T)).astype(np.float32),
        "w1t": np.stack([swz_dh(w1[e].T) for e in range(E)]).astype(bf),
        "w3t": np.stack([swz_dh(w3[e].T) for e in range(E)]).astype(bf),
        "w2t": np.stack([swz_hd(w2[e].T) for e in range(E)]).astype(bf),
        "w1st": swz_dh(w1s.T).astype(bf),
        "w3st": swz_dh(w3s.T).astype(bf),
        "w2st": swz_hd(w2s.T).astype(bf),
        "biasb": np.tile(expert_bias.astype(np.float32), (P, 1)),
        "iotab": np.tile(np.arange(E, dtype=np.float32), (P, 1)),
        "ebasem1": np.tile((np.arange(E) * CAP - 1).astype(np.float32), (P, 1)),
        "triu": np.triu(np.ones((P, P), dtype=np.float32)),
        "trils": np.tril(np.ones((P, P), dtype=np.float32), k=-1),
    }
    in_maps = []
    for j in range(NCORES):
        xs = x[j * TPC:(j + 1) * TPC]
        if xs.shape[0] == 0:
            continue
        m = dict(shared)
        xsw = np.ascontiguousarray(
            xs.reshape(NT, P, DJ, P).transpose(0, 3, 2, 1))
        m["xt32"] = xsw.astype(np.float32)
        m["xbf"] = np.ascontiguousarray(xs).astype(bf)
        m["xtbf"] = xsw.astype(bf)
        in_maps.append(m)
    return in_maps


def kernel(x, gate_w, w1, w2, w3, w1s, w2s, w3s, expert_bias, _trace=False):
    x = np.asarray(x)
    in_maps = _prep_inputs(np.asarray(x, np.float32), np.asarray(gate_w),
                           np.asarray(w1), np.asarray(w2), np.asarray(w3),
                           np.asarray(w1s), np.asarray(w2s), np.asarray(w3s),
                           np.asarray(expert_bias))
    if "nc" not in _cache:
        _cache["nc"] = build_bass()
    res = bass_utils.run_bass_kernel_spmd(
        _cache["nc"], in_maps, core_ids=list(range(NCORES)), trace=_trace)
    out = np.concatenate([r["out"] for r in res.results], axis=0)
    _cache["last_results"] = res
    return out.astype(np.float32)
